# revision 1
# baseline (speedup 1.0000x reference)
"""KNN-Attention Trainium2 kernel (Bass/Tile), SPMD over 8 NeuronCores.

Problem (nn_KNNAttention): B=2, H=8, S=2048, D=64, K=32.
  q:[B,H,S,D] k,v:[B,S,D] mask:[B,S] mem_k,mem_v:[B,H,S,K,D]
  mem_mask:[B,H,S,K] rel_pos_bias:[1,H,S,S] scale:[H,1,1]
  out[b,h,i,:] = softmax([sim_mem | sim_local]) @ [mem_v | v]

Sharding: data-parallel over B x tensor-parallel over H.
core c -> (b = c//4, heads 2*(c%4), 2*(c%4)+1). k/v/mask replicated per b.

Per-core dataflow (2 heads x 16 i-tiles of 128 tokens):
  - l2norm(k) once -> kT [64, 2048] resident (PE transpose).
  - v' = [v*mask | mask] bf16 resident; the extra column yields the local
    softmax denominator from the same matmul that computes attn@v.
  - per (head, i-tile):
      qs = q * exp(scale)/||q||  (scale folded into q)
      scores = qsT.T @ kT (fp32 PE) -> +bias (DVE) -> exp (ACT, bf16 out)
      exp blocks PE-transposed -> AV matmul (bf16) accumulates [i, 65] psum
      mem: prod = mem_k*qs (GPSIMD) -> seg-reduce d (DVE) -> exp (ACT)
           prod2 = mem_v*exp_mem (GPSIMD/DVE split) -> seg-reduce kk (DVE)
      out = (local_num + mem_num) / (local_den + mem_den)
  - causal handled by only computing j<=i blocks; the upper triangle of the
    diagonal bias blocks is set to -FLT_MAX host-side (exp -> 0, exact).
"""

import os
import sys
from contextlib import ExitStack

import numpy as np

sys.path.insert(0, "/opt/trn_rl_repo")

import concourse.bass as bass
import concourse.mybir as mybir
import concourse.tile as tile
from concourse import bacc

# Keep all ACT functions in ONE table set (natural_log_exp_and_others holds
# Exp+Ln+Copy+Identity) so the kernel pays a single ACT_TABLE_LOAD instead of
# swapping sets every iteration. Other sets keep their dict position (the
# act_func_set_id is positional) but lose the overlapping functions, forcing
# the selector to the combined set.
_orig_get_act_tables = bacc.get_activation_tables
_PREF_SET = "natural_log_exp_and_others"


def _uni_act_tables(arch):
    tabs = _orig_get_act_tables(arch)
    if _PREF_SET in tabs:
        pref = tabs[_PREF_SET]
        for name, funcs in tabs.items():
            if name != _PREF_SET:
                tabs[name] = funcs - pref
    return tabs


bacc.get_activation_tables = _uni_act_tables
from concourse.bass_utils import run_bass_kernel_spmd

B, H, S, D, KK = 2, 8, 2048, 64, 32
P = 128
NT = S // P  # 16 i-tiles
NH = 2  # heads per core
N_CORES = 8
NEG = -np.finfo(np.float32).max
import ml_dtypes

IDENT_F = np.eye(P, dtype=np.float32)
IDENT_B = np.eye(P, dtype=np.float32).astype(ml_dtypes.bfloat16)

F32 = mybir.dt.float32
BF16 = mybir.dt.bfloat16
U8 = mybir.dt.uint8
AX = mybir.AxisListType
ALU = mybir.AluOpType
ACTF = mybir.ActivationFunctionType


def build_program(nh=NH, nt=NT):
    """Build the per-core Bass program (SPMD: same program, different data)."""
    nc = bacc.Bacc("TRN2")
    s = nt * P

    q_d = nc.dram_tensor("q", [nh, s, D], F32, kind="ExternalInput")
    k_d = nc.dram_tensor("k", [s, D], F32, kind="ExternalInput")
    v_d = nc.dram_tensor("v", [s, D], F32, kind="ExternalInput")
    mask_d = nc.dram_tensor("mask", [s], F32, kind="ExternalInput")
    memk_d = nc.dram_tensor("mem_k", [nh, s, KK, D], F32, kind="ExternalInput")
    memv_d = nc.dram_tensor("mem_v", [nh, s, KK, D], F32, kind="ExternalInput")
    mmask_d = nc.dram_tensor("mem_mask", [nh, s, KK], U8, kind="ExternalInput")
    bias_d = nc.dram_tensor("bias", [nh, s, s], BF16, kind="ExternalInput")
    scale_d = nc.dram_tensor("scale", [nh], F32, kind="ExternalInput")
    identf_d = nc.dram_tensor("ident_f", [P, P], F32, kind="ExternalInput")
    identb_d = nc.dram_tensor("ident_b", [P, P], BF16, kind="ExternalInput")
    out_d = nc.dram_tensor("out", [nh, s, D], F32, kind="ExternalOutput")

    with tile.TileContext(nc) as tc, ExitStack() as ctx:
        const = ctx.enter_context(tc.tile_pool(name="const", bufs=1))
        setup = ctx.enter_context(tc.tile_pool(name="setup", bufs=3))
        qpool = ctx.enter_context(tc.tile_pool(name="qpool", bufs=4))
        stream = ctx.enter_context(tc.tile_pool(name="stream", bufs=3))
        work = ctx.enter_context(tc.tile_pool(name="work", bufs=6))
        expTp = ctx.enter_context(tc.tile_pool(name="expTp", bufs=8))
        memw = ctx.enter_context(tc.tile_pool(name="memw", bufs=3))
        smallw = ctx.enter_context(tc.tile_pool(name="smallw", bufs=8))
        outp = ctx.enter_context(tc.tile_pool(name="outp", bufs=2))
        ps_sco = ctx.enter_context(tc.tile_pool(name="ps_sco", bufs=2, space="PSUM"))
        ps_tp_f = ctx.enter_context(tc.tile_pool(name="ps_tp_f", bufs=2, space="PSUM"))
        ps_tp_b = ctx.enter_context(tc.tile_pool(name="ps_tp_b", bufs=2, space="PSUM"))
        ps_u = ctx.enter_context(tc.tile_pool(name="ps_u", bufs=2, space="PSUM"))

        # ---- constants (DMA'd: keeps PE instruction wait lists short) ----
        ident_f = const.tile([P, P], F32)
        nc.sync.dma_start(out=ident_f, in_=identf_d[:])
        ident_b = const.tile([P, P], BF16)
        nc.sync.dma_start(out=ident_b, in_=identb_d[:])

        # ---- sc[h] = exp(scale[h]) broadcast to [P,1] per head via DMA ----
        sc_b = const.tile([P, nh], F32)
        sc_raw = const.tile([P, nh], F32)
        nc.sync.dma_start(
            out=sc_raw, in_=scale_d[None, :].to_broadcast((P, nh))
        )
        nc.scalar.activation(sc_b, sc_raw, ACTF.Exp)

        # ---- k: l2norm, transpose -> kT [64, s]; v' = [v*mask | mask] bf16 ----
        kT_stage = const.tile([D, s], F32)
        kT = const.tile([D, s], F32)
        v_bf = const.tile([P, nt, D + 1], BF16)
        for jt in range(nt):
            k_t = setup.tile([P, D], F32, tag="k_t")
            nc.sync.dma_start(out=k_t, in_=k_d[jt * P : (jt + 1) * P, :])
            ksq = setup.tile([P, D], F32, tag="ksq")
            nc.vector.tensor_mul(ksq, k_t, k_t)
            ksum = setup.tile([P, 1], F32, tag="ksum")
            nc.vector.tensor_reduce(ksum, ksq, axis=AX.X, op=ALU.add)
            kln = setup.tile([P, 1], F32, tag="kln")
            nc.scalar.activation(kln, ksum, ACTF.Ln)
            # rsqrt(sumsq) = exp(-0.5*ln(sumsq)); Ln+Exp share one ACT table set
            rk = setup.tile([P, 1], F32, tag="rk")
            nc.scalar.activation(rk, kln, ACTF.Exp, scale=-0.5)
            kn = setup.tile([P, D], F32, tag="kn")
            nc.vector.tensor_scalar_mul(kn, k_t, rk)
            ps_k = ps_tp_f.tile([D, P], F32, tag="tpf")
            nc.tensor.transpose(ps_k, kn, ident_f)
            nc.scalar.copy(kT_stage[:, jt * P : (jt + 1) * P], ps_k)
        # single-writer consolidation so matmuls reading kT wait on one proc
        nc.vector.tensor_copy(kT, kT_stage)

        # v' built with two instructions total (writer-count discipline)
        v_sb = const.tile([P, nt, D], F32)
        nc.sync.dma_start(
            out=v_sb, in_=v_d[:].rearrange("(t p) d -> p t d", p=P)
        )
        m_sb = const.tile([P, nt], F32)
        nc.sync.dma_start(out=m_sb, in_=mask_d[:].rearrange("(t p) -> p t", p=P))
        nc.vector.tensor_tensor(
            v_bf[:, :, 0:D], v_sb, m_sb[:, :, None].to_broadcast((P, nt, D)), ALU.mult
        )
        nc.vector.tensor_copy(v_bf[:, :, D], m_sb)

        # ---- main loop ----
        for h in range(nh):
            out_acc = outp.tile([P, nt, D], F32, tag="out_acc")
            for it in range(nt):
                jext = (it + 1) * P
                # q tile: l2norm and fold in sc
                q_t = qpool.tile([P, D], F32, tag="q_t")
                nc.sync.dma_start(out=q_t, in_=q_d[h, it * P : (it + 1) * P, :])
                qsq = qpool.tile([P, D], F32, tag="qsq")
                nc.gpsimd.tensor_mul(qsq, q_t, q_t)
                qsum = qpool.tile([P, 1], F32, tag="qsum")
                nc.vector.tensor_reduce(qsum, qsq, axis=AX.X, op=ALU.add)
                qln = qpool.tile([P, 1], F32, tag="qln")
                nc.scalar.activation(qln, qsum, ACTF.Ln)
                rq = qpool.tile([P, 1], F32, tag="rq")
                nc.scalar.activation(rq, qln, ACTF.Exp, scale=-0.5)
                sc_rq = qpool.tile([P, 1], F32, tag="sc_rq")
                nc.vector.tensor_mul(sc_rq, rq, sc_b[:, h : h + 1])
                qs = qpool.tile([P, D], F32, tag="qs")
                nc.vector.tensor_scalar_mul(qs, q_t, sc_rq)
                ps_q = ps_tp_f.tile([D, P], F32, tag="tpf")
                nc.tensor.transpose(ps_q, qs, ident_f)
                qT = qpool.tile([D, P], F32, tag="qT")
                nc.scalar.copy(qT, ps_q)

                # streamed tiles
                memk = stream.tile([P, KK, D], F32, tag="memk")
                nc.sync.dma_start(out=memk, in_=memk_d[h, it * P : (it + 1) * P])
                memv = stream.tile([P, KK, D], F32, tag="memv")
                nc.sync.dma_start(out=memv, in_=memv_d[h, it * P : (it + 1) * P])
                mmask = stream.tile([P, KK], U8, tag="mmask")
                nc.sync.dma_start(out=mmask, in_=mmask_d[h, it * P : (it + 1) * P])
                bias_t = stream.tile([P, S], BF16, tag="bias_t")
                nc.sync.dma_start(
                    out=bias_t[:, :jext],
                    in_=bias_d[h, it * P : (it + 1) * P, 0:jext],
                )

                # ---- knn-memory branch ----
                prod = memw.tile([P, KK, D], F32, tag="prod")
                nc.gpsimd.tensor_tensor(
                    prod, memk, qs[:, None, :].to_broadcast((P, KK, D)), ALU.mult
                )
                simmem = smallw.tile([P, KK], F32, tag="simmem")
                nc.vector.tensor_reduce(simmem, prod, axis=AX.X, op=ALU.add)
                # joint-softmax stabilizer: M = max(rowmax(sim_mem), 21) covers
                # the unnormalized mem logits (~N(0,20)); local logits are
                # bounded by 20+|bias| < 21, so exp(l - M) never overflows.
                rowmax = smallw.tile([P, 1], F32, tag="rowmax")
                nc.vector.tensor_reduce(rowmax, simmem, axis=AX.X, op=ALU.max)
                negM = smallw.tile([P, 1], F32, tag="negM")
                nc.vector.tensor_scalar(
                    negM, rowmax, 21.0, -1.0, ALU.max, ALU.mult
                )
                expmem = smallw.tile([P, KK], F32, tag="expmem")
                nc.scalar.activation(expmem, simmem, ACTF.Exp, bias=negM)
                mmf = smallw.tile([P, KK], F32, tag="mmf")
                nc.gpsimd.tensor_copy(mmf, mmask)
                nc.gpsimd.tensor_mul(expmem, expmem, mmf)
                zmem = smallw.tile([P, 1], F32, tag="zmem")
                nc.vector.tensor_reduce(zmem, expmem, axis=AX.X, op=ALU.add)
                prod2 = memw.tile([P, D, KK], F32, tag="prod2")
                p2w = prod2[:].rearrange("p d k -> p k d")
                eb = expmem[:, :, None].to_broadcast((P, KK, D))
                nc.gpsimd.tensor_tensor(p2w, memv, eb, ALU.mult)
                memout = smallw.tile([P, D], F32, tag="memout")
                nc.vector.tensor_reduce(memout, prod2, axis=AX.X, op=ALU.add)

                # ---- local branch ----
                psum_u = ps_u.tile([P, D + 1], F32, tag="u")
                for j0 in range(0, jext, 512):
                    w = min(512, jext - j0)
                    ps_s = ps_sco.tile([P, 512], F32, tag="sco")
                    nc.tensor.matmul(
                        ps_s[:, :w],
                        lhsT=qT,
                        rhs=kT[:, j0 : j0 + w],
                        start=True,
                        stop=True,
                    )
                    expb0 = work.tile([P, 512], BF16, tag="expb0")
                    nc.scalar.activation(expb0[:, :w], ps_s[:, :w], ACTF.Exp, bias=negM)
                    expb = work.tile([P, 512], BF16, tag="expb")
                    nc.vector.tensor_mul(
                        expb[:, :w], expb0[:, :w], bias_t[:, j0 : j0 + w]
                    )
                    for jj in range(0, w, P):
                        jt_g = (j0 + jj) // P
                        ps_t = ps_tp_b.tile([P, P], BF16, tag="tpb")
                        nc.tensor.transpose(ps_t, expb[:, jj : jj + P], ident_b)
                        eT = expTp.tile([P, P], BF16, tag="eT")
                        nc.scalar.copy(eT, ps_t)
                        nc.tensor.matmul(
                            psum_u,
                            lhsT=eT,
                            rhs=v_bf[:, jt_g, :],
                            start=(jt_g == 0),
                            stop=(jt_g == it),
                        )

                # ---- combine ----
                num = smallw.tile([P, D], F32, tag="num")
                nc.vector.tensor_add(num, psum_u[:, 0:D], memout)
                z = smallw.tile([P, 1], F32, tag="z")
                nc.vector.tensor_add(z, psum_u[:, D : D + 1], zmem)
                rz = smallw.tile([P, 1], F32, tag="rz")
                nc.vector.reciprocal(rz, z)
                nc.vector.tensor_scalar_mul(out_acc[:, it, :], num, rz)

            nc.sync.dma_start(
                out=out_d[h].rearrange("(t p) d -> p t d", p=P), in_=out_acc
            )

    nc.compile()
    return nc


_CACHED = {}
TRACE = False
TRACE_CORES = [0]
STITCH = False
LAST_RESULTS = None


def _get_program(nh=NH, nt=NT):
    key = (nh, nt)
    if key not in _CACHED:
        _CACHED[key] = build_program(nh, nt)
    return _CACHED[key]


def _merge_causal(bias):
    """bias: [H, S, S] float32 (a copy). Set upper triangle of each diagonal
    128-block to -FLT_MAX. Off-diagonal upper blocks are never read."""
    iu = np.triu_indices(P, 1)
    for t in range(S // P):
        blk = bias[:, t * P : (t + 1) * P, t * P : (t + 1) * P]
        blk[:, iu[0], iu[1]] = NEG
    return bias


def kernel(**inputs):
    q = np.ascontiguousarray(inputs["q"], dtype=np.float32)
    k = np.ascontiguousarray(inputs["k"], dtype=np.float32)
    v = np.ascontiguousarray(inputs["v"], dtype=np.float32)
    mask = np.ascontiguousarray(inputs["mask"], dtype=np.float32)
    mem_k = np.ascontiguousarray(inputs["mem_k"], dtype=np.float32)
    mem_v = np.ascontiguousarray(inputs["mem_v"], dtype=np.float32)
    mem_mask = np.ascontiguousarray(inputs["mem_mask"]).astype(np.uint8)
    rel_pos_bias = np.array(inputs["rel_pos_bias"], dtype=np.float32)
    scale = np.ascontiguousarray(inputs["scale"], dtype=np.float32).reshape(H)

    bias = _merge_causal(rel_pos_bias.reshape(H, S, S).copy())
    bias = np.exp(bias).astype(ml_dtypes.bfloat16)

    nc = _get_program()
    in_maps = []
    for c in range(N_CORES):
        b = c // 4
        h0 = 2 * (c % 4)
        in_maps.append(
            {
                "q": np.ascontiguousarray(q[b, h0 : h0 + NH]),
                "k": k[b],
                "v": v[b],
                "mask": mask[b],
                "mem_k": np.ascontiguousarray(mem_k[b, h0 : h0 + NH]),
                "mem_v": np.ascontiguousarray(mem_v[b, h0 : h0 + NH]),
                "mem_mask": np.ascontiguousarray(mem_mask[b, h0 : h0 + NH]),
                "bias": np.ascontiguousarray(bias[h0 : h0 + NH]),
                "scale": np.ascontiguousarray(scale[h0 : h0 + NH]),
                "ident_f": IDENT_F,
                "ident_b": IDENT_B,
            }
        )

    global LAST_RESULTS
    kwargs = {}
    if TRACE:
        kwargs.update(trace=True, trace_cores=TRACE_CORES, stitch_traces=STITCH)
    res = run_bass_kernel_spmd(nc, in_maps, core_ids=list(range(N_CORES)), **kwargs)
    LAST_RESULTS = res

    out = np.zeros((B, H, S, D), np.float32)
    for c in range(N_CORES):
        b = c // 4
        h0 = 2 * (c % 4)
        out[b, h0 : h0 + NH] = res.results[c]["out"]
    return out


if __name__ == "__main__":
    # smoke test via CoreSim on a reduced config
    from concourse.bass_interp import CoreSim

    nh, nt = int(os.environ.get("SMOKE_NH","1")), int(os.environ.get("SMOKE_NT","2"))
    s = nt * P
    rng = np.random.default_rng(0)
    qs = rng.standard_normal((nh, s, D), dtype=np.float32)
    ks = rng.standard_normal((s, D), dtype=np.float32)
    vs = rng.standard_normal((s, D), dtype=np.float32)
    ms = np.ones((s,), np.float32)
    mks = rng.standard_normal((nh, s, KK, D), dtype=np.float32)
    mvs = rng.standard_normal((nh, s, KK, D), dtype=np.float32)
    mms = np.ones((nh, s, KK), np.uint8)
    bs = (rng.standard_normal((nh, s, s)) * 0.02).astype(np.float32)
    scs = np.full((nh,), np.log(20.0), np.float32)

    # numpy reference for the reduced problem
    def ref():
        qq = qs / np.linalg.norm(qs, axis=-1, keepdims=True)
        kk_ = ks / np.linalg.norm(ks, axis=-1, keepdims=True)
        sc = np.exp(scs)[:, None, None]
        sim = np.einsum("hid,jd->hij", qq, kk_) * sc + bs
        causal = np.triu(np.ones((s, s), bool), 1)
        sim = np.where(causal[None], NEG, sim)
        simm = np.einsum("hid,hijd->hij", qq, mks) * sc
        att = np.concatenate([simm, sim], axis=-1)
        att = att - att.max(-1, keepdims=True)
        att = np.exp(att)
        att = att / att.sum(-1, keepdims=True)
        mem_a, loc_a = att[..., :KK], att[..., KK:]
        return np.einsum("hij,jd->hid", loc_a, vs) + np.einsum(
            "hij,hijd->hid", mem_a, mvs
        )

    bias_s = bs.copy()
    iu = np.triu_indices(P, 1)
    for t in range(nt):
        blk = bias_s[:, t * P : (t + 1) * P, t * P : (t + 1) * P]
        blk[:, iu[0], iu[1]] = NEG
    bias_s = np.exp(bias_s).astype(ml_dtypes.bfloat16)

    nc = build_program(nh, nt)
    sim = CoreSim(nc)
    for name, val in [
        ("q", qs), ("k", ks), ("v", vs), ("mask", ms), ("mem_k", mks),
        ("mem_v", mvs), ("mem_mask", mms), ("bias", bias_s), ("scale", scs),
        ("ident_f", IDENT_F), ("ident_b", IDENT_B),
    ]:
        sim.tensor(name)[:] = val
    sim.simulate()
    got = np.array(sim.tensor("out")).reshape(nh, s, D)
    exp = ref()
    err = np.abs(got - exp).max() / np.abs(exp).max()
    print("abs-rel err:", err)
    assert err < 2e-2, err
    print("CoreSim smoke PASSED")

